# revision 11
# baseline (speedup 1.0000x reference)
"""Trainium2 Bass kernel for nn_C_Encoder_78434692760248.

Computation (per row of x [N=262144, D=256]):
    h = x
    for i in 0..3:  h = tanh(LN(h; g1[i], b1[i]) @ W1[i].T + b1v[i])
    out = concat(LN(h; g2, b2) @ W2.T + bv2,  LN(h; g3, b3) @ W3.T + bv3)

Strategy: pure data-parallel over 8 NeuronCores (32768 rows each).
Host-side algebra folds each "LN affine + Linear" into a single matmul
against pre-transformed weights:
  - LN gain/bias fold:  V = W * g[None,:],  b_eff = b + W @ beta
  - z = (h - mu) * rstd has exact zero row-sum, so feature 255 is
    eliminated (Vt[d] = V[:,d] - V[:,255] for d<255), freeing one
    contraction slot; a constant-1 column there carries b_eff through
    the matmul. Contraction stays exactly 256 = 2x128 PE chunks.
Device pipeline is row-major [128 rows x 256 feat] tiles, fp16 matmul
operands (error ~1e-3 vs fp32 reference), fp32 stats/psum:
  DMA x -> bn_stats/bn_aggr -> rstd=(var+eps)^-0.5 (DVE pow)
  -> tensor_scalar normalize (fp16 z, ones col) -> transpose (PE or
  DMA-xbar, configurable split) -> 2 accumulating matmuls vs resident
  fp16 weights -> batched tanh on ScalarE -> ... -> fp16 out, host
  upcasts to fp32.
"""

import sys

import numpy as np

sys.path.insert(0, "/opt/trn_rl_repo")

EPS = 1e-5
N_CORES = 8
N_ROWS = 262144
D = 256
P = 128
TILES_PER_BLOCK = 4  # 512 rows per block
ROWS_PER_CORE = N_ROWS // N_CORES

# --- tuning knobs (engine assignment) --------------------------------------
# per LN-pass index 0..4: engine for the normalize tensor_scalar
NORM_ENGINE = ["vector", "vector", "vector", "vector", "vector"]
# per LN-pass index 0..4: how z gets transposed ("pe" or "dma").
# NOTE: "dma" (xbar transpose) triggers Tile's xbar-mode serialization,
# which adds sync waits to every other DMA and overflows the HWDGE
# 1-wait limit in walrus codegen. Keep "pe" unless that is solved.
TRANSPOSE_PATH = ["dma", "dma", "dma", "dma", "dma"]
# engine for the PSUM->SBUF copy after PE transposes ("scalar" or "vector")
COPYBACK_ENGINE = "scalar"
# engine for final psum -> out staging copy
OUTCOPY_ENGINE = "scalar"


def _fold_weights(inputs):
    """Fold LN affines + linear layers into 6 matmul matrices [256,256] fp16.

    Rt[l][i, j]: rows 0..254 = V[j,i] - V[j,255]; row 255 = b_eff[j].
    Streamed against zbuf rows [z_0..z_254, 1].
    """
    W1, b1 = np.asarray(inputs["W1"], np.float32), np.asarray(inputs["b1"], np.float32)
    g1, be1 = np.asarray(inputs["ln1_g"], np.float32), np.asarray(inputs["ln1_b"], np.float32)
    mats = []
    for i in range(4):
        mats.append((W1[i], b1[i], g1[i], be1[i]))
    mats.append((np.asarray(inputs["W2"], np.float32), np.asarray(inputs["b2"], np.float32),
                 np.asarray(inputs["ln2_g"], np.float32), np.asarray(inputs["ln2_b"], np.float32)))
    mats.append((np.asarray(inputs["W3"], np.float32), np.asarray(inputs["b3"], np.float32),
                 np.asarray(inputs["ln3_g"], np.float32), np.asarray(inputs["ln3_b"], np.float32)))
    Rts = []
    for W, b, g, beta in mats:
        V = W * g[None, :]
        b_eff = b + W @ beta
        Vt = (V[:, :255] - V[:, 255:256]).T            # [255, 256]
        Rt = np.concatenate([Vt, b_eff[None, :]], 0)   # [256, 256]
        Rts.append(Rt)
    Rt_all = np.stack(Rts, 0).reshape(6, 2, 128, 256).astype(np.float16)
    return Rt_all  # [6, 2, 128, 256]


def _build_program(rows_per_core):
    """Build the Bass/Tile program. Same program runs SPMD on all cores."""
    import concourse.bass as bass
    import concourse.tile as tile
    from concourse import bacc, mybir
    from concourse.bass import ts
    from concourse.masks import make_identity

    f32 = mybir.dt.float32
    f16 = mybir.dt.float16
    Alu = mybir.AluOpType
    Act = mybir.ActivationFunctionType

    n_blocks = rows_per_core // (P * TILES_PER_BLOCK)
    assert n_blocks * P * TILES_PER_BLOCK == rows_per_core

    nc = bacc.Bacc("TRN2", target_bir_lowering=False)
    x_d = nc.dram_tensor("x", [rows_per_core, D], f32, kind="ExternalInput")
    rt_d = nc.dram_tensor("Rt", [6, 2, 128, 256], f16, kind="ExternalInput")
    out_d = nc.dram_tensor("out", [rows_per_core, 2 * D], f16, kind="ExternalOutput")

    x_v = x_d.rearrange("(B t p) d -> B p t d", t=TILES_PER_BLOCK, p=P)
    out_v = out_d.rearrange("(B t p) j -> B p t j", t=TILES_PER_BLOCK, p=P)

    norm_eng = {"vector": nc.vector, "gpsimd": nc.gpsimd}
    T = TILES_PER_BLOCK

    with tile.TileContext(nc) as tc:
        with (
            tc.tile_pool(name="singles", bufs=1) as singles,
            tc.tile_pool(name="xp", bufs=3) as xp,
            tc.tile_pool(name="hp", bufs=3) as hp,
            tc.tile_pool(name="zp", bufs=3) as zp,
            tc.tile_pool(name="ztp", bufs=3) as ztp,
            tc.tile_pool(name="stat", bufs=3) as stat,
            tc.tile_pool(name="ost", bufs=3) as ost,
            tc.tile_pool(name="yps", bufs=2, space="PSUM") as yps_pool,
            tc.tile_pool(name="ztps", bufs=2, space="PSUM") as ztps_pool,
            tc.tile_pool(name="fps", bufs=2, space="PSUM") as fps_pool,
        ):
            # --- resident constants ---
            rt_sb = singles.tile([P, 6, 2, 256], f16)
            nc.sync.dma_start(out=rt_sb, in_=rt_d.rearrange("l c i j -> i l c j"))
            ident = singles.tile([P, P], f16)
            make_identity(nc, ident)

            for b in range(n_blocks):
                x_t = xp.tile([P, T, D], f32)
                nc.sync.dma_start(out=x_t, in_=x_v[b])
                src = x_t

                ostage = ost.tile([P, T, 2 * D], f16)

                for li in range(5):  # LN passes: 0..3 feed tanh layers, 4 feeds branches
                    # --- stats ---
                    bst = stat.tile([P, T, 6], f32, tag="bst")
                    for t in range(T):
                        nc.vector.bn_stats(out=bst[:, t, :], in_=src[:, t, :])
                    mv = stat.tile([P, T, 2], f32, tag="mv")
                    for t in range(T):
                        nc.vector.bn_aggr(out=mv[:, t, :], in_=bst[:, t, :])
                    # rstd = (var+eps)^-0.5 on DVE only: hardware has no
                    # pow/rsqrt (ACT Sqrt lives in a different table set than
                    # Tanh -> 2.7us switches). Quake bit-hack seed + 2 Newton
                    # iterations, rel err ~1e-6.
                    i32 = mybir.dt.int32
                    v = stat.tile([P, T], f32, tag="v")
                    nc.vector.tensor_scalar(
                        out=v, in0=mv[:, :, 1], scalar1=EPS, scalar2=None,
                        op0=Alu.add,
                    )
                    rstd = stat.tile([P, T], f32, tag="rstd")
                    nc.vector.tensor_scalar(
                        out=rstd.bitcast(i32), in0=v.bitcast(i32),
                        scalar1=1, scalar2=None, op0=Alu.arith_shift_right,
                    )
                    nc.vector.tensor_scalar(
                        out=rstd.bitcast(i32), in0=rstd.bitcast(i32),
                        scalar1=-1, scalar2=0x5F3759DF, op0=Alu.mult, op1=Alu.add,
                    )
                    tN = stat.tile([P, T], f32, tag="tN")
                    for _ in range(2):  # Newton: y = y*(1.5 - 0.5*v*y^2)
                        nc.vector.tensor_tensor(out=tN, in0=rstd, in1=rstd, op=Alu.mult)
                        nc.vector.tensor_tensor(out=tN, in0=tN, in1=v, op=Alu.mult)
                        nc.vector.tensor_scalar(
                            out=tN, in0=tN, scalar1=-0.5, scalar2=1.5,
                            op0=Alu.mult, op1=Alu.add,
                        )
                        nc.vector.tensor_tensor(out=rstd, in0=rstd, in1=tN, op=Alu.mult)
                    # --- normalize into zbuf (fp16), ones column at 255 ---
                    z = zp.tile([P, T, D], f16)
                    neng = norm_eng[NORM_ENGINE[li]]
                    # ones col written by the same engine as the normalize so
                    # downstream transposes see a single producer engine
                    neng.memset(z[:, :, 255:256], 1.0)
                    for t in range(T):
                        neng.tensor_scalar(
                            out=z[:, t, 0:255], in0=src[:, t, 0:255],
                            scalar1=mv[:, t, 0:1], scalar2=rstd[:, t : t + 1],
                            op0=Alu.subtract, op1=Alu.mult,
                        )
                    # --- transpose z -> zT [P, 2T, 128] (chunk-major per tile) ---
                    z_t = ztp.tile([P, 2 * T, P], f16)
                    if TRANSPOSE_PATH[li] == "pe":
                        zps = ztps_pool.tile([P, 2 * T, P], f16)
                        for t in range(T):
                            for c in range(2):
                                nc.tensor.transpose(
                                    zps[:, 2 * t + c, :], z[:, t, ts(c, P)], ident
                                )
                        if COPYBACK_ENGINE == "scalar":
                            nc.scalar.copy(out=z_t, in_=zps)
                        else:
                            nc.vector.tensor_copy(out=z_t, in_=zps)
                    else:
                        for t in range(T):
                            for c in range(2):
                                nc.scalar.dma_start_transpose(
                                    z_t[:, 2 * t + c, :], z[:, t, ts(c, P)]
                                )
                    # --- matmuls ---
                    if li < 4:
                        y = yps_pool.tile([P, T, D], f32)
                        for t in range(T):
                            nc.tensor.matmul(
                                y[:, t, :], z_t[:, 2 * t, :], rt_sb[:, li, 0, :],
                                start=True, stop=False,
                            )
                            nc.tensor.matmul(
                                y[:, t, :], z_t[:, 2 * t + 1, :], rt_sb[:, li, 1, :],
                                start=False, stop=True,
                            )
                        h = hp.tile([P, T, D], f16)
                        nc.scalar.activation(out=h, in_=y, func=Act.Tanh)
                        src = h
                    else:
                        for t in range(T):
                            f = fps_pool.tile([P, 2, D], f32)
                            for br in range(2):  # mean branch (l=4), var branch (l=5)
                                nc.tensor.matmul(
                                    f[:, br, :], z_t[:, 2 * t, :], rt_sb[:, 4 + br, 0, :],
                                    start=True, stop=False,
                                )
                                nc.tensor.matmul(
                                    f[:, br, :], z_t[:, 2 * t + 1, :], rt_sb[:, 4 + br, 1, :],
                                    start=False, stop=True,
                                )
                            if OUTCOPY_ENGINE == "scalar":
                                nc.scalar.copy(out=ostage[:, t, :], in_=f)
                            else:
                                nc.vector.tensor_copy(out=ostage[:, t, :], in_=f)
                nc.sync.dma_start(out=out_v[b], in_=ostage)
    nc.compile()
    return nc


_PROGRAM_CACHE = {}


def _get_program(rows_per_core):
    if rows_per_core not in _PROGRAM_CACHE:
        _PROGRAM_CACHE[rows_per_core] = _build_program(rows_per_core)
    return _PROGRAM_CACHE[rows_per_core]


def run(inputs, trace=False, **spmd_kwargs):
    """Shard, run on 8 cores, gather. Returns (full_output, BassKernelResults)."""
    from concourse.bass_utils import run_bass_kernel_spmd

    x = np.asarray(inputs["x"], np.float32)
    n = x.shape[0]
    rows_per_core = n // N_CORES
    Rt = _fold_weights(inputs)

    nc = _get_program(rows_per_core)
    in_maps = [
        {"x": np.ascontiguousarray(x[i * rows_per_core : (i + 1) * rows_per_core]),
         "Rt": Rt}
        for i in range(N_CORES)
    ]
    res = run_bass_kernel_spmd(nc, in_maps, list(range(N_CORES)), trace=trace,
                               **spmd_kwargs)
    outs = [np.asarray(res.results[i]["out"], np.float32) for i in range(N_CORES)]
    return np.concatenate(outs, 0), res


def kernel(**inputs):
    out, _ = run(inputs)
    return out


if __name__ == "__main__":
    # smoke-build only
    nc = _build_program(1024)
    print("build ok:", len(nc.instructions) if hasattr(nc, "instructions") else "nc ready")


# revision 12
# speedup vs baseline: 2.0257x; 2.0257x over previous
"""Trainium2 Bass kernel for nn_C_Encoder_78434692760248.

Computation (per row of x [N=262144, D=256]):
    h = x
    for i in 0..3:  h = tanh(LN(h; g1[i], b1[i]) @ W1[i].T + b1v[i])
    out = concat(LN(h; g2, b2) @ W2.T + bv2,  LN(h; g3, b3) @ W3.T + bv3)

Strategy: pure data-parallel over 8 NeuronCores (32768 rows each).
Host-side algebra folds each "LN affine + Linear" into a single matmul
against pre-transformed weights:
  - LN gain/bias fold:  V = W * g[None,:],  b_eff = b + W @ beta
  - z = (h - mu) * rstd has exact zero row-sum, so feature 255 is
    eliminated (Vt[d] = V[:,d] - V[:,255] for d<255), freeing one
    contraction slot; a constant-1 column there carries b_eff through
    the matmul. Contraction stays exactly 256 = 2x128 PE chunks.
Device pipeline is row-major [128 rows x 256 feat] tiles, fp16 matmul
operands (error ~1e-3 vs fp32 reference), fp32 stats/psum:
  DMA x -> bn_stats/bn_aggr -> rstd=(var+eps)^-0.5 (DVE pow)
  -> tensor_scalar normalize (fp16 z, ones col) -> transpose (PE or
  DMA-xbar, configurable split) -> 2 accumulating matmuls vs resident
  fp16 weights -> batched tanh on ScalarE -> ... -> fp16 out, host
  upcasts to fp32.
"""

import sys

import numpy as np

sys.path.insert(0, "/opt/trn_rl_repo")

EPS = 1e-5
N_CORES = 8
N_ROWS = 262144
D = 256
P = 128
TILES_PER_BLOCK = 4  # 512 rows per block
ROWS_PER_CORE = N_ROWS // N_CORES

# --- tuning knobs (engine assignment) --------------------------------------
# per LN-pass index 0..4: engine for the normalize tensor_scalar
NORM_ENGINE = ["vector", "vector", "vector", "vector", "vector"]
# per LN-pass index 0..4: how z gets transposed ("pe" or "dma").
# NOTE: "dma" (xbar transpose) triggers Tile's xbar-mode serialization,
# which adds sync waits to every other DMA and overflows the HWDGE
# 1-wait limit in walrus codegen. Keep "pe" unless that is solved.
TRANSPOSE_PATH = ["pe", "pe", "pe", "pe", "pe"]
# engine for the PSUM->SBUF copy after PE transposes ("scalar" or "vector")
COPYBACK_ENGINE = "scalar"
# engine for final psum -> out staging copy
OUTCOPY_ENGINE = "scalar"


def _fold_weights(inputs):
    """Fold LN affines + linear layers into 6 matmul matrices [256,256] fp16.

    Rt[l][i, j]: rows 0..254 = V[j,i] - V[j,255]; row 255 = b_eff[j].
    Streamed against zbuf rows [z_0..z_254, 1].
    """
    W1, b1 = np.asarray(inputs["W1"], np.float32), np.asarray(inputs["b1"], np.float32)
    g1, be1 = np.asarray(inputs["ln1_g"], np.float32), np.asarray(inputs["ln1_b"], np.float32)
    mats = []
    for i in range(4):
        mats.append((W1[i], b1[i], g1[i], be1[i]))
    mats.append((np.asarray(inputs["W2"], np.float32), np.asarray(inputs["b2"], np.float32),
                 np.asarray(inputs["ln2_g"], np.float32), np.asarray(inputs["ln2_b"], np.float32)))
    mats.append((np.asarray(inputs["W3"], np.float32), np.asarray(inputs["b3"], np.float32),
                 np.asarray(inputs["ln3_g"], np.float32), np.asarray(inputs["ln3_b"], np.float32)))
    Rts = []
    for W, b, g, beta in mats:
        V = W * g[None, :]
        b_eff = b + W @ beta
        Vt = (V[:, :255] - V[:, 255:256]).T            # [255, 256]
        Rt = np.concatenate([Vt, b_eff[None, :]], 0)   # [256, 256]
        Rts.append(Rt)
    Rt_all = np.stack(Rts, 0).reshape(6, 2, 128, 256).astype(np.float16)
    return Rt_all  # [6, 2, 128, 256]


def _build_program(rows_per_core):
    """Build the Bass/Tile program. Same program runs SPMD on all cores."""
    import concourse.bass as bass
    import concourse.tile as tile
    from concourse import bacc, mybir
    from concourse.bass import ts
    from concourse.masks import make_identity

    f32 = mybir.dt.float32
    f16 = mybir.dt.float16
    Alu = mybir.AluOpType
    Act = mybir.ActivationFunctionType

    n_blocks = rows_per_core // (P * TILES_PER_BLOCK)
    assert n_blocks * P * TILES_PER_BLOCK == rows_per_core

    nc = bacc.Bacc("TRN2", target_bir_lowering=False)
    x_d = nc.dram_tensor("x", [rows_per_core, D], f32, kind="ExternalInput")
    rt_d = nc.dram_tensor("Rt", [6, 2, 128, 256], f16, kind="ExternalInput")
    out_d = nc.dram_tensor("out", [rows_per_core, 2 * D], f16, kind="ExternalOutput")

    x_v = x_d.rearrange("(B t p) d -> B p t d", t=TILES_PER_BLOCK, p=P)
    out_v = out_d.rearrange("(B t p) j -> B p t j", t=TILES_PER_BLOCK, p=P)

    norm_eng = {"vector": nc.vector, "gpsimd": nc.gpsimd}
    T = TILES_PER_BLOCK

    with tile.TileContext(nc) as tc:
        with (
            tc.tile_pool(name="singles", bufs=1) as singles,
            tc.tile_pool(name="xp", bufs=3) as xp,
            tc.tile_pool(name="hp", bufs=3) as hp,
            tc.tile_pool(name="zp", bufs=3) as zp,
            tc.tile_pool(name="ztp", bufs=3) as ztp,
            tc.tile_pool(name="stat", bufs=3) as stat,
            tc.tile_pool(name="ost", bufs=3) as ost,
            tc.tile_pool(name="yps", bufs=2, space="PSUM") as yps_pool,
            tc.tile_pool(name="ztps", bufs=2, space="PSUM") as ztps_pool,
            tc.tile_pool(name="fps", bufs=2, space="PSUM") as fps_pool,
        ):
            # --- resident constants ---
            rt_sb = singles.tile([P, 6, 2, 256], f16)
            nc.sync.dma_start(out=rt_sb, in_=rt_d.rearrange("l c i j -> i l c j"))
            ident = singles.tile([P, P], f16)
            make_identity(nc, ident)

            for b in range(n_blocks):
                x_t = xp.tile([P, T, D], f32)
                nc.sync.dma_start(out=x_t, in_=x_v[b])
                src = x_t

                ostage = ost.tile([P, T, 2 * D], f16)

                for li in range(5):  # LN passes: 0..3 feed tanh layers, 4 feeds branches
                    # --- stats ---
                    bst = stat.tile([P, T, 6], f32, tag="bst")
                    for t in range(T):
                        nc.vector.bn_stats(out=bst[:, t, :], in_=src[:, t, :])
                    mv = stat.tile([P, T, 2], f32, tag="mv")
                    for t in range(T):
                        nc.vector.bn_aggr(out=mv[:, t, :], in_=bst[:, t, :])
                    # rstd = (var+eps)^-0.5 on DVE only: hardware has no
                    # pow/rsqrt (ACT Sqrt lives in a different table set than
                    # Tanh -> 2.7us switches). Quake bit-hack seed + 2 Newton
                    # iterations, rel err ~1e-6.
                    i32 = mybir.dt.int32
                    v = stat.tile([P, T], f32, tag="v")
                    nc.vector.tensor_scalar(
                        out=v, in0=mv[:, :, 1], scalar1=EPS, scalar2=None,
                        op0=Alu.add,
                    )
                    rstd = stat.tile([P, T], f32, tag="rstd")
                    nc.vector.tensor_scalar(
                        out=rstd.bitcast(i32), in0=v.bitcast(i32),
                        scalar1=1, scalar2=None, op0=Alu.arith_shift_right,
                    )
                    nc.vector.tensor_scalar(
                        out=rstd.bitcast(i32), in0=rstd.bitcast(i32),
                        scalar1=-1, scalar2=0x5F3759DF, op0=Alu.mult, op1=Alu.add,
                    )
                    tN = stat.tile([P, T], f32, tag="tN")
                    for _ in range(2):  # Newton: y = y*(1.5 - 0.5*v*y^2)
                        nc.vector.tensor_tensor(out=tN, in0=rstd, in1=rstd, op=Alu.mult)
                        nc.vector.tensor_tensor(out=tN, in0=tN, in1=v, op=Alu.mult)
                        nc.vector.tensor_scalar(
                            out=tN, in0=tN, scalar1=-0.5, scalar2=1.5,
                            op0=Alu.mult, op1=Alu.add,
                        )
                        nc.vector.tensor_tensor(out=rstd, in0=rstd, in1=tN, op=Alu.mult)
                    # --- normalize into zbuf (fp16), ones column at 255 ---
                    z = zp.tile([P, T, D], f16)
                    neng = norm_eng[NORM_ENGINE[li]]
                    # ones col written by the same engine as the normalize so
                    # downstream transposes see a single producer engine
                    neng.memset(z[:, :, 255:256], 1.0)
                    for t in range(T):
                        neng.tensor_scalar(
                            out=z[:, t, 0:255], in0=src[:, t, 0:255],
                            scalar1=mv[:, t, 0:1], scalar2=rstd[:, t : t + 1],
                            op0=Alu.subtract, op1=Alu.mult,
                        )
                    # --- transpose z -> zT [P, 2T, 128] (chunk-major per tile) ---
                    z_t = ztp.tile([P, 2 * T, P], f16)
                    if TRANSPOSE_PATH[li] == "pe":
                        zps = ztps_pool.tile([P, 2 * T, P], f16)
                        for t in range(T):
                            for c in range(2):
                                nc.tensor.transpose(
                                    zps[:, 2 * t + c, :], z[:, t, ts(c, P)], ident
                                )
                        if COPYBACK_ENGINE == "scalar":
                            nc.scalar.copy(out=z_t, in_=zps)
                        else:
                            nc.vector.tensor_copy(out=z_t, in_=zps)
                    else:
                        for t in range(T):
                            for c in range(2):
                                nc.scalar.dma_start_transpose(
                                    z_t[:, 2 * t + c, :], z[:, t, ts(c, P)]
                                )
                    # --- matmuls ---
                    if li < 4:
                        y = yps_pool.tile([P, T, D], f32)
                        for t in range(T):
                            nc.tensor.matmul(
                                y[:, t, :], z_t[:, 2 * t, :], rt_sb[:, li, 0, :],
                                start=True, stop=False,
                            )
                            nc.tensor.matmul(
                                y[:, t, :], z_t[:, 2 * t + 1, :], rt_sb[:, li, 1, :],
                                start=False, stop=True,
                            )
                        h = hp.tile([P, T, D], f16)
                        nc.scalar.activation(out=h, in_=y, func=Act.Tanh)
                        src = h
                    else:
                        for t in range(T):
                            f = fps_pool.tile([P, 2, D], f32)
                            for br in range(2):  # mean branch (l=4), var branch (l=5)
                                nc.tensor.matmul(
                                    f[:, br, :], z_t[:, 2 * t, :], rt_sb[:, 4 + br, 0, :],
                                    start=True, stop=False,
                                )
                                nc.tensor.matmul(
                                    f[:, br, :], z_t[:, 2 * t + 1, :], rt_sb[:, 4 + br, 1, :],
                                    start=False, stop=True,
                                )
                            if OUTCOPY_ENGINE == "scalar":
                                nc.scalar.copy(out=ostage[:, t, :], in_=f)
                            else:
                                nc.vector.tensor_copy(out=ostage[:, t, :], in_=f)
                nc.sync.dma_start(out=out_v[b], in_=ostage)
    nc.compile()
    return nc


_PROGRAM_CACHE = {}


def _get_program(rows_per_core):
    if rows_per_core not in _PROGRAM_CACHE:
        _PROGRAM_CACHE[rows_per_core] = _build_program(rows_per_core)
    return _PROGRAM_CACHE[rows_per_core]


def run(inputs, trace=False, **spmd_kwargs):
    """Shard, run on 8 cores, gather. Returns (full_output, BassKernelResults)."""
    from concourse.bass_utils import run_bass_kernel_spmd

    x = np.asarray(inputs["x"], np.float32)
    n = x.shape[0]
    rows_per_core = n // N_CORES
    Rt = _fold_weights(inputs)

    nc = _get_program(rows_per_core)
    in_maps = [
        {"x": np.ascontiguousarray(x[i * rows_per_core : (i + 1) * rows_per_core]),
         "Rt": Rt}
        for i in range(N_CORES)
    ]
    res = run_bass_kernel_spmd(nc, in_maps, list(range(N_CORES)), trace=trace,
                               **spmd_kwargs)
    outs = [np.asarray(res.results[i]["out"], np.float32) for i in range(N_CORES)]
    return np.concatenate(outs, 0), res


def kernel(**inputs):
    out, _ = run(inputs)
    return out


if __name__ == "__main__":
    # smoke-build only
    nc = _build_program(1024)
    print("build ok:", len(nc.instructions) if hasattr(nc, "instructions") else "nc ready")


# revision 15
# speedup vs baseline: 3.3537x; 1.6556x over previous
"""Trainium2 Bass kernel for nn_C_Encoder_78434692760248.

Computation (per row of x [N=262144, D=256]):
    h = x
    for i in 0..3:  h = tanh(LN(h; g1[i], b1[i]) @ W1[i].T + b1v[i])
    out = concat(LN(h; g2, b2) @ W2.T + bv2,  LN(h; g3, b3) @ W3.T + bv3)

Strategy: pure data-parallel over 8 NeuronCores (32768 rows each).
Host-side algebra folds each "LN affine + Linear" into a single matmul
against pre-transformed weights:
  - LN gain/bias fold:  V = W * g[None,:],  b_eff = b + W @ beta
  - z = (h - mu) * rstd has exact zero row-sum, so feature 255 is
    eliminated (Vt[d] = V[:,d] - V[:,255] for d<255), freeing one
    contraction slot; a constant-1 column there carries b_eff through
    the matmul. Contraction stays exactly 256 = 2x128 PE chunks.
Device pipeline is row-major [128 rows x 256 feat] tiles, fp16 matmul
operands (error ~1e-3 vs fp32 reference), fp32 stats/psum:
  DMA x -> bn_stats/bn_aggr -> rstd=(var+eps)^-0.5 (DVE pow)
  -> tensor_scalar normalize (fp16 z, ones col) -> transpose (PE or
  DMA-xbar, configurable split) -> 2 accumulating matmuls vs resident
  fp16 weights -> batched tanh on ScalarE -> ... -> fp16 out, host
  upcasts to fp32.
"""

import sys

import numpy as np

sys.path.insert(0, "/opt/trn_rl_repo")

EPS = 1e-5
N_CORES = 8
N_ROWS = 262144
D = 256
P = 128
TILES_PER_BLOCK = 4  # 512 rows per block
ROWS_PER_CORE = N_ROWS // N_CORES

# --- tuning knobs (engine assignment) --------------------------------------
# per LN-pass index 0..4: engine for the normalize tensor_scalar
NORM_ENGINE = ["vector", "vector", "vector", "vector", "vector"]
# per LN-pass index 0..4: how z gets transposed ("pe" or "dma").
# NOTE: "dma" (xbar transpose) triggers Tile's xbar-mode serialization,
# which adds sync waits to every other DMA and overflows the HWDGE
# 1-wait limit in walrus codegen. Keep "pe" unless that is solved.
TRANSPOSE_PATH = ["pe", "pe", "pe", "pe", "pe"]
# engine for the PSUM->SBUF copy after PE transposes ("scalar" or "vector")
COPYBACK_ENGINE = "scalar"
# engine for final psum -> out staging copy
OUTCOPY_ENGINE = "scalar"
# number of block-chains interleaved in emission order (software pipelining)
N_CHAINS = 2


def _fold_weights(inputs):
    """Fold LN affines + linear layers into 6 matmul matrices [256,256] fp16.

    Rt[l][i, j]: rows 0..254 = V[j,i] - V[j,255]; row 255 = b_eff[j].
    Streamed against zbuf rows [z_0..z_254, 1].
    """
    W1, b1 = np.asarray(inputs["W1"], np.float32), np.asarray(inputs["b1"], np.float32)
    g1, be1 = np.asarray(inputs["ln1_g"], np.float32), np.asarray(inputs["ln1_b"], np.float32)
    mats = []
    for i in range(4):
        mats.append((W1[i], b1[i], g1[i], be1[i]))
    mats.append((np.asarray(inputs["W2"], np.float32), np.asarray(inputs["b2"], np.float32),
                 np.asarray(inputs["ln2_g"], np.float32), np.asarray(inputs["ln2_b"], np.float32)))
    mats.append((np.asarray(inputs["W3"], np.float32), np.asarray(inputs["b3"], np.float32),
                 np.asarray(inputs["ln3_g"], np.float32), np.asarray(inputs["ln3_b"], np.float32)))
    Rts = []
    for W, b, g, beta in mats:
        V = W * g[None, :]
        b_eff = b + W @ beta
        Vt = (V[:, :255] - V[:, 255:256]).T            # [255, 256]
        Rt = np.concatenate([Vt, b_eff[None, :]], 0)   # [256, 256]
        Rts.append(Rt)
    Rt_all = np.stack(Rts, 0).reshape(6, 2, 128, 256).astype(np.float16)
    return Rt_all  # [6, 2, 128, 256]


def _build_program(rows_per_core):
    """Build the Bass/Tile program. Same program runs SPMD on all cores."""
    import concourse.bass as bass
    import concourse.tile as tile
    from concourse import bacc, mybir
    from concourse.bass import ts
    from concourse.masks import make_identity

    f32 = mybir.dt.float32
    f16 = mybir.dt.float16
    Alu = mybir.AluOpType
    Act = mybir.ActivationFunctionType

    n_blocks = rows_per_core // (P * TILES_PER_BLOCK)
    assert n_blocks * P * TILES_PER_BLOCK == rows_per_core

    nc = bacc.Bacc("TRN2", target_bir_lowering=False)
    x_d = nc.dram_tensor("x", [rows_per_core, D], f32, kind="ExternalInput")
    rt_d = nc.dram_tensor("Rt", [6, 2, 128, 256], f16, kind="ExternalInput")
    out_d = nc.dram_tensor("out", [rows_per_core, 2 * D], f16, kind="ExternalOutput")

    x_v = x_d.rearrange("(B t p) d -> B p t d", t=TILES_PER_BLOCK, p=P)
    out_v = out_d.rearrange("(B t p) j -> B p t j", t=TILES_PER_BLOCK, p=P)

    norm_eng = {"vector": nc.vector, "gpsimd": nc.gpsimd}
    T = TILES_PER_BLOCK

    with tile.TileContext(nc) as tc:
        with (
            tc.tile_pool(name="singles", bufs=1) as singles,
            tc.tile_pool(name="xp", bufs=3) as xp,
            tc.tile_pool(name="hp", bufs=3) as hp,
            tc.tile_pool(name="zp", bufs=3) as zp,
            tc.tile_pool(name="ztp", bufs=3) as ztp,
            tc.tile_pool(name="stat", bufs=3) as stat,
            tc.tile_pool(name="ost", bufs=3) as ost,
            tc.tile_pool(name="yps", bufs=2, space="PSUM") as yps_pool,
            tc.tile_pool(name="ztps", bufs=2, space="PSUM") as ztps_pool,
            tc.tile_pool(name="fps", bufs=2, space="PSUM") as fps_pool,
        ):
            # --- resident constants ---
            rt_sb = singles.tile([P, 6, 2, 256], f16)
            nc.sync.dma_start(out=rt_sb, in_=rt_d.rearrange("l c i j -> i l c j"))
            ident = singles.tile([P, P], f16)
            make_identity(nc, ident)

            i32 = mybir.dt.int32

            # --- stage emitters over a per-chain state dict -------------
            def st_load(ch):
                x_t = xp.tile([P, T, D], f32, tag="x", name=f"x_{ch['b']}")
                nc.sync.dma_start(out=x_t, in_=x_v[ch["b"]])
                ch["src"] = x_t
                ch["ostage"] = ost.tile([P, T, 2 * D], f16, tag="ostage", name=f"ost_{ch['b']}")

            def st_stats(ch, li):
                src = ch["src"]
                bst = stat.tile([P, T, 6], f32, tag="bst", name=f"bst_{ch['b']}_{li}")
                for t in range(T):
                    nc.vector.bn_stats(out=bst[:, t, :], in_=src[:, t, :])
                mv = stat.tile([P, T, 2], f32, tag="mv", name=f"mv_{ch['b']}_{li}")
                for t in range(T):
                    nc.vector.bn_aggr(out=mv[:, t, :], in_=bst[:, t, :])
                # rstd = (var+eps)^-0.5: no HW pow/rsqrt (ACT Sqrt is in a
                # different table set than Tanh -> 2.7us switches). Quake
                # bit-hack seed + 2 Newton iterations, rel err ~1e-6.
                v = stat.tile([P, T], f32, tag="v", name=f"v_{ch['b']}_{li}")
                nc.vector.tensor_scalar(
                    out=v, in0=mv[:, :, 1], scalar1=EPS, scalar2=None, op0=Alu.add
                )
                rstd = stat.tile([P, T], f32, tag="rstd", name=f"rstd_{ch['b']}_{li}")
                nc.vector.tensor_scalar(
                    out=rstd.bitcast(i32), in0=v.bitcast(i32),
                    scalar1=1, scalar2=None, op0=Alu.arith_shift_right,
                )
                nc.vector.tensor_scalar(
                    out=rstd.bitcast(i32), in0=rstd.bitcast(i32),
                    scalar1=-1, scalar2=0x5F3759DF, op0=Alu.mult, op1=Alu.add,
                )
                tN = stat.tile([P, T], f32, tag="tN", name=f"tN_{ch['b']}_{li}")
                for _ in range(2):  # Newton: y = y*(1.5 - 0.5*v*y^2)
                    nc.vector.tensor_tensor(out=tN, in0=rstd, in1=rstd, op=Alu.mult)
                    nc.vector.tensor_tensor(out=tN, in0=tN, in1=v, op=Alu.mult)
                    nc.vector.tensor_scalar(
                        out=tN, in0=tN, scalar1=-0.5, scalar2=1.5,
                        op0=Alu.mult, op1=Alu.add,
                    )
                    nc.vector.tensor_tensor(out=rstd, in0=rstd, in1=tN, op=Alu.mult)
                ch["mv"], ch["rstd"] = mv, rstd

            def st_norm(ch, li):
                src, mv, rstd = ch["src"], ch["mv"], ch["rstd"]
                z = zp.tile([P, T, D], f16, tag="z", name=f"z_{ch['b']}_{li}")
                neng = norm_eng[NORM_ENGINE[li]]
                neng.memset(z[:, :, 255:256], 1.0)
                for t in range(T):
                    neng.tensor_scalar(
                        out=z[:, t, 0:255], in0=src[:, t, 0:255],
                        scalar1=mv[:, t, 0:1], scalar2=rstd[:, t : t + 1],
                        op0=Alu.subtract, op1=Alu.mult,
                    )
                ch["z"] = z

            def st_transpose(ch, li):
                z = ch["z"]
                z_t = ztp.tile([P, 2 * T, P], f16, tag="zt", name=f"zt_{ch['b']}_{li}")
                zps = ztps_pool.tile([P, 2 * T, P], f16, tag="zps", name=f"zps_{ch['b']}_{li}")
                for t in range(T):
                    for c in range(2):
                        nc.tensor.transpose(
                            zps[:, 2 * t + c, :], z[:, t, ts(c, P)], ident
                        )
                if COPYBACK_ENGINE == "scalar":
                    nc.scalar.copy(out=z_t, in_=zps)
                else:
                    nc.vector.tensor_copy(out=z_t, in_=zps)
                ch["z_t"] = z_t

            def st_matmul(ch, li):
                z_t = ch["z_t"]
                if li < 4:
                    y = yps_pool.tile([P, T, D], f32, tag="y", name=f"y_{ch['b']}_{li}")
                    for t in range(T):
                        nc.tensor.matmul(
                            y[:, t, :], z_t[:, 2 * t, :], rt_sb[:, li, 0, :],
                            start=True, stop=False,
                        )
                        nc.tensor.matmul(
                            y[:, t, :], z_t[:, 2 * t + 1, :], rt_sb[:, li, 1, :],
                            start=False, stop=True,
                        )
                    h = hp.tile([P, T, D], f16, tag="h", name=f"h_{ch['b']}_{li}")
                    nc.scalar.activation(out=h, in_=y, func=Act.Tanh)
                    ch["src"] = h
                else:
                    ostage = ch["ostage"]
                    for t in range(T):
                        f = fps_pool.tile([P, 2, D], f32, tag="f", name=f"f_{ch['b']}_{t}")
                        for br in range(2):  # mean (l=4), var (l=5) branches
                            nc.tensor.matmul(
                                f[:, br, :], z_t[:, 2 * t, :], rt_sb[:, 4 + br, 0, :],
                                start=True, stop=False,
                            )
                            nc.tensor.matmul(
                                f[:, br, :], z_t[:, 2 * t + 1, :], rt_sb[:, 4 + br, 1, :],
                                start=False, stop=True,
                            )
                        if OUTCOPY_ENGINE == "scalar":
                            nc.scalar.copy(out=ostage[:, t, :], in_=f)
                        else:
                            nc.vector.tensor_copy(out=ostage[:, t, :], in_=f)

            def st_store(ch):
                nc.sync.dma_start(out=out_v[ch["b"]], in_=ch["ostage"])

            # --- N_CHAINS independent block-chains, stages interleaved so
            # each engine always has ready work from another chain while one
            # chain crosses an engine-hop latency -------------------------
            for bp in range(0, n_blocks, N_CHAINS):
                chains = [{"b": bp + k} for k in range(min(N_CHAINS, n_blocks - bp))]
                for ch in chains:
                    st_load(ch)
                for li in range(5):
                    for ch in chains:
                        st_stats(ch, li)
                    for ch in chains:
                        st_norm(ch, li)
                    for ch in chains:
                        st_transpose(ch, li)
                    for ch in chains:
                        st_matmul(ch, li)
                for ch in chains:
                    st_store(ch)
    nc.compile()
    return nc


_PROGRAM_CACHE = {}


def _get_program(rows_per_core):
    if rows_per_core not in _PROGRAM_CACHE:
        _PROGRAM_CACHE[rows_per_core] = _build_program(rows_per_core)
    return _PROGRAM_CACHE[rows_per_core]


def run(inputs, trace=False, **spmd_kwargs):
    """Shard, run on 8 cores, gather. Returns (full_output, BassKernelResults)."""
    from concourse.bass_utils import run_bass_kernel_spmd

    x = np.asarray(inputs["x"], np.float32)
    n = x.shape[0]
    rows_per_core = n // N_CORES
    Rt = _fold_weights(inputs)

    nc = _get_program(rows_per_core)
    in_maps = [
        {"x": np.ascontiguousarray(x[i * rows_per_core : (i + 1) * rows_per_core]),
         "Rt": Rt}
        for i in range(N_CORES)
    ]
    res = run_bass_kernel_spmd(nc, in_maps, list(range(N_CORES)), trace=trace,
                               **spmd_kwargs)
    outs = [np.asarray(res.results[i]["out"], np.float32) for i in range(N_CORES)]
    return np.concatenate(outs, 0), res


def kernel(**inputs):
    out, _ = run(inputs)
    return out


if __name__ == "__main__":
    # smoke-build only
    nc = _build_program(1024)
    print("build ok:", len(nc.instructions) if hasattr(nc, "instructions") else "nc ready")
